# revision 1
# baseline (speedup 1.0000x reference)
"""Trainium2 Bass kernel for nn_CrossAttention_90400471646744.

Reference math (B=8, NQ=77, NK=128, D=512, H=8, DH=64):
    q    = (x @ Wq)                          # [b, nq, d]
    k    = (context @ Wk)                    # [b, nk, d]
    to_v = (x @ Wv).reshape(b, nq, d, d)     # per-query value projection
    v    = einsum('bkd,bqde->bqke', context, to_v)
    sim  = einsum per head of q.k / sqrt(dh)
    attn = softmax(sim)
    out  = (einsum('bhqk,bhqkd->bhqd', attn, v) merged) @ Wo

Key algebraic refactor (avoids the 617MB to_v / v intermediates):
    out_pre[b,q,e] = sum_d1 A[b,h(e),q,d1] * to_v[b,q,d1,e]
    where A = attn @ context  ([b,h,q,d1])
so we never form v. The huge compute is to_v = x @ Wv (165 GFLOP total).

Sharding: the d1 axis (512) is split across the 8 cores; core i takes
d1 in [i*64, (i+1)*64), i.e. the contiguous Wv column block
[i*32768, (i+1)*32768).  Each core computes a partial out (including
the Wo projection, which is linear) and the host sums the 8 partials.

Per-core dataflow (rows r = b*77+q, 616 rows in 5 row tiles):
  preamble: qT/kT projections, per-(b,h) sim matmul + exp + row-sum +
    PE transpose, E = attnT.T @ context_d1slice  (softmax 1/sum folded
    into the E copy via a per-partition activation scale), then E is
    repartitioned from [q, .] to row-major [b*77+q, .] layout via
    SBUF->SBUF DMA.
  main: for each d1 group (8 x 8 d1): stream Wv tiles, T = x @ Wv_tile
    (PE, fp32r), M = T * broadcast(E)  (DVE, stride-0 AP), segmented
    reduce over d1 accumulating out_pre per row tile.
  tail: PE-transpose out_pre, out^T = Wo^T-tiles @ out_pre^T, DMA out.

kernel(**inputs) takes the FULL inputs and returns the FULL output.
"""

import contextlib
import numpy as np

import concourse.bass as bass
import concourse.bacc as bacc
import concourse.tile as tile
from concourse import mybir
from concourse.bass_utils import run_bass_kernel_spmd

F32 = mybir.dt.float32
F32R = mybir.dt.float32r
ADD = mybir.AluOpType.add
MULT = mybir.AluOpType.mult
AX_X = mybir.AxisListType.X
EXP = mybir.ActivationFunctionType.Exp
COPY = mybir.ActivationFunctionType.Copy

B, NQ, NK, D, H = 8, 77, 128, 512, 8
DH = D // H                      # 64
ROWS = B * NQ                    # 616
N_CORES = 8
DSH = D // N_CORES               # 64 d1 values per core
WCOLS = DSH * D                  # 32768 Wv cols per core
CT = D // 128                    # 4 c(ontraction) tiles
GD = 8                           # d1 per group
NG = DSH // GD                   # 8 groups
# row tiles (offset, length); 616 = 4*128 + 104
RT = [(0, 128), (128, 128), (256, 128), (384, 128), (512, 104)]
# q/k/out row chunks for N<=512 moving limit
RCHUNKS = [(0, 308), (308, 308)]


def _b_segments(b):
    """Split rows b*77..b*77+77 into (q0, block, p0, len) with constant
    128-partition block — used to repartition [q, .] -> [row, .]."""
    segs = []
    q = 0
    while q < NQ:
        r = b * NQ + q
        blk, p = divmod(r, 128)
        ln = min(NQ - q, 128 - p)
        segs.append((q, blk, p, ln))
        q += ln
    return segs


def build_program(reps=1):
    nc = bacc.Bacc("TRN2", target_bir_lowering=False, debug=False,
                   num_devices=N_CORES)

    xT_d = nc.dram_tensor("xT", [D, ROWS], F32R, kind="ExternalInput")
    ctxT_d = nc.dram_tensor("ctxT", [D, B * NK], F32R, kind="ExternalInput")
    ctxd1_d = nc.dram_tensor("ctxd1", [B * NK, DSH], F32, kind="ExternalInput")
    wq_d = nc.dram_tensor("Wq", [D, D], F32R, kind="ExternalInput")
    wk_d = nc.dram_tensor("Wk", [D, D], F32R, kind="ExternalInput")
    wo_d = nc.dram_tensor("Wo", [D, D], F32R, kind="ExternalInput")
    wv_d = nc.dram_tensor("Wv_s", [D, WCOLS], F32R, kind="ExternalInput")
    outT_d = nc.dram_tensor("outT", [D, ROWS], F32, kind="ExternalOutput")
    ident_d = nc.inline_tensor(np.eye(128, dtype=np.float32), name="ident")

    def r32(ap):
        return ap.bitcast(F32R)

    with tile.TileContext(nc) as tc:
        with (
            tc.For_i(0, reps, 1) if reps > 1 else contextlib.nullcontext(),
            tc.tile_pool(name="const", bufs=1) as cp,
        ):
            xT = cp.tile([128, CT * ROWS], F32R, tag="xT")
            for ct in range(CT):
                nc.sync.dma_start(xT[:, ct * ROWS:(ct + 1) * ROWS],
                                  xT_d[ct * 128:(ct + 1) * 128, :])
            wo = cp.tile([128, CT * D], F32R, tag="wo")
            for et in range(CT):
                nc.sync.dma_start(wo[:, et * D:(et + 1) * D],
                                  wo_d[et * 128:(et + 1) * 128, :])
            ident = cp.tile([128, 128], F32, tag="ident")
            nc.sync.dma_start(ident[:], ident_d[:])
            # E in row-major layout: [row % 128, (row//128)*512 + h*64 + d1]
            E_sb = cp.tile([128, len(RT) * D], F32, tag="E")

            # wv pool opens before the preamble so group 0 streams in
            # concurrently with the attention preamble
            _wv_cm = tc.tile_pool(name="wv", bufs=44)
            wvp = _wv_cm.__enter__()
            wvt = {}
            for dj in range(GD):
                for ct in range(CT):
                    t = wvp.tile([128, 512], F32R, tag="wv",
                                 name=f"wv0_{dj}_{ct}")
                    nc.sync.dma_start(
                        t[:], wv_d[ct * 128:(ct + 1) * 128,
                                   dj * 512:(dj + 1) * 512])
                    wvt[(0, dj, ct)] = t

            # ---------------- preamble: attention ----------------
            with (
                tc.tile_pool(name="pre", bufs=1) as pp,
                tc.tile_pool(name="pre2", bufs=3) as pp2,
                tc.tile_pool(name="est", bufs=2) as estp,
                tc.tile_pool(name="pre_ps", bufs=2, space="PSUM") as pps,
                tc.tile_pool(name="sim_ps", bufs=2, space="PSUM") as sps,
                tc.tile_pool(name="at_ps", bufs=2, space="PSUM") as aps,
                tc.tile_pool(name="e_ps", bufs=2, space="PSUM") as eps,
            ):
                wq = pp.tile([128, CT * D], F32R, tag="wq")
                wk = pp.tile([128, CT * D], F32R, tag="wk")
                ctxT = pp.tile([128, CT * B * NK], F32R, tag="ctxT")
                ctxd1 = pp.tile([128, B * DSH], F32, tag="ctxd1")
                for ct in range(CT):
                    nc.sync.dma_start(wq[:, ct * D:(ct + 1) * D],
                                      wq_d[ct * 128:(ct + 1) * 128, :])
                    nc.sync.dma_start(wk[:, ct * D:(ct + 1) * D],
                                      wk_d[ct * 128:(ct + 1) * 128, :])
                    nc.sync.dma_start(
                        ctxT[:, ct * B * NK:(ct + 1) * B * NK],
                        ctxT_d[ct * 128:(ct + 1) * 128, :])
                for b in range(B):
                    nc.sync.dma_start(ctxd1[:, b * DSH:(b + 1) * DSH],
                                      ctxd1_d[b * NK:(b + 1) * NK, :])

                qT = pp.tile([128, CT * ROWS], F32, tag="qT")
                kT = pp.tile([128, CT * B * NK], F32, tag="kT")
                # projections: qT[m, r] = sum_c Wq[c, m] * xT[c, r]
                for mt in range(CT):
                    for (co, cl) in RCHUNKS:
                        ps = pps.tile([128, 512], F32, tag="qkps")
                        for ct in range(CT):
                            nc.tensor.matmul(
                                ps[:, :cl],
                                wq[:, ct * D + mt * 128:
                                    ct * D + mt * 128 + 128],
                                xT[:, ct * ROWS + co:ct * ROWS + co + cl],
                                start=(ct == 0), stop=(ct == CT - 1))
                        nc.vector.tensor_copy(
                            qT[:, mt * ROWS + co:mt * ROWS + co + cl],
                            ps[:, :cl])
                    for ko in range(0, B * NK, 512):
                        ps = pps.tile([128, 512], F32, tag="qkps")
                        for ct in range(CT):
                            nc.tensor.matmul(
                                ps[:],
                                wk[:, ct * D + mt * 128:
                                    ct * D + mt * 128 + 128],
                                ctxT[:, ct * B * NK + ko:
                                      ct * B * NK + ko + 512],
                                start=(ct == 0), stop=(ct == CT - 1))
                        nc.vector.tensor_copy(
                            kT[:, mt * B * NK + ko:mt * B * NK + ko + 512],
                            ps[:])

                rsum = pp.tile([128, B * H], F32, tag="rsum")
                rrec = pp.tile([128, B * H], F32, tag="rrec")
                for b in range(B):
                    est = estp.tile([128, D], F32, tag="est")
                    for h in range(H):
                        bh = b * H + h
                        pb = 64 * (h % 2)
                        mt = h // 2
                        q_sl = qT[pb:pb + 64,
                                  mt * ROWS + b * NQ:mt * ROWS + b * NQ + NQ]
                        k_sl = kT[pb:pb + 64,
                                  mt * B * NK + b * NK:
                                  mt * B * NK + b * NK + NK]
                        sim = sps.tile([NQ, NK], F32, tag="sim")
                        nc.tensor.matmul(sim[:], q_sl, k_sl)
                        expt = pp2.tile([NQ, NK], F32, tag="exp")
                        # scale = dh**-0.5 folded into the exp argument;
                        # accum_out gives the softmax denominator for free
                        nc.scalar.activation(expt[:], sim[:], EXP,
                                             scale=float(DH) ** -0.5,
                                             accum_out=rsum[:NQ, bh:bh + 1])
                        nc.vector.reciprocal(rrec[:NQ, bh:bh + 1],
                                             rsum[:NQ, bh:bh + 1])
                        atp = aps.tile([128, NQ], F32, tag="at")
                        nc.tensor.transpose(atp[:, :NQ], expt[:],
                                            ident[:NQ, :NQ])
                        at = pp2.tile([128, NQ], F32, tag="atsb")
                        nc.vector.tensor_copy(at[:], atp[:, :NQ])
                        ep = eps.tile([NQ, DSH], F32, tag="ep")
                        nc.tensor.matmul(ep[:], at[:],
                                         ctxd1[:, b * DSH:(b + 1) * DSH])
                        # 1/rowsum folded in here (per-partition scalar);
                        # on DVE so ACT stays Exp-only (no table thrash)
                        nc.vector.tensor_scalar_mul(
                            est[:NQ, h * DH:(h + 1) * DH], ep[:],
                            rrec[:NQ, bh:bh + 1])
                    # repartition [q, (h,d1)] -> row-major E_sb
                    for (q0, blk, p0, ln) in _b_segments(b):
                        nc.sync.dma_start(
                            E_sb[p0:p0 + ln, blk * D:(blk + 1) * D],
                            est[q0:q0 + ln, :])

            # ---------------- main loop ----------------
            with tc.tile_pool(name="acc", bufs=1) as accp:
              acc = [accp.tile([128, D], F32, tag=f"acc{i}",
                               name=f"acc{i}")
                     for i in range(len(RT))]
              with (
                tc.tile_pool(name="tmp", bufs=3) as tmpp,
                tc.tile_pool(name="t_ps", bufs=2, space="PSUM") as tps,
              ):
                for g in range(NG):
                    if g > 0:
                        for dj in range(GD):
                            d1 = g * GD + dj
                            for ct in range(CT):
                                t = wvp.tile([128, 512], F32R, tag="wv",
                                             name=f"wv{g}_{dj}_{ct}")
                                nc.sync.dma_start(
                                    t[:],
                                    wv_d[ct * 128:(ct + 1) * 128,
                                         d1 * 512:(d1 + 1) * 512])
                                wvt[(g, dj, ct)] = t
                    for irt, (ro, rl) in enumerate(RT):
                        for pk in range(GD // 4):
                            dj0 = 4 * pk
                            d1 = g * GD + dj0
                            # 4-bank PSUM tile: four d1 values side by side
                            t_ps = tps.tile([128, 2048], F32, tag="T")
                            for q4 in range(4):
                                for ct in range(CT):
                                    nc.tensor.matmul(
                                        t_ps[:rl, q4 * 512:
                                             q4 * 512 + 512],
                                        xT[:, ct * ROWS + ro:
                                           ct * ROWS + ro + rl],
                                        wvt[(g, dj0 + q4, ct)][:],
                                        start=(ct == 0),
                                        stop=(ct == CT - 1))
                            # coeff[p, dj4, h, dh] = E_sb[r, irt*512
                            #                          + h*64 + (d1+dj4)]
                            ebc = (E_sb[:rl, irt * D:(irt + 1) * D]
                                   .rearrange("p (h dh) -> p h dh", dh=DH)
                                   [:, :, d1:d1 + 4]
                                   .transpose((0, 2, 1))
                                   .to_broadcast((rl, 4, H, DH)))
                            in0 = t_ps[:rl, :].rearrange(
                                "p (dj4 h dh) -> p dj4 h dh",
                                dj4=4, dh=DH)
                            tmp = tmpp.tile([128, 2048], F32, tag="tmp")
                            tv = tmp[:rl, :].rearrange(
                                "p (dj4 h dh) -> p dj4 h dh", dj4=4, dh=DH)
                            nc.vector.tensor_tensor(tv, in0, ebc, op=MULT)
                            for k in range(4):
                                sl = tmp[:rl, k * 512:k * 512 + 512]
                                if g == 0 and pk == 0 and k == 0:
                                    continue
                                if g == 0 and pk == 0 and k == 1:
                                    nc.vector.tensor_tensor(
                                        acc[irt][:rl, :],
                                        tmp[:rl, 0:512], sl, op=ADD)
                                    continue
                                # 2 adds per (g, rt) on DVE, 6 on GPSIMD
                                eng = (nc.vector if (pk == 0 and k < 2)
                                       else nc.gpsimd)
                                eng.tensor_tensor(acc[irt][:rl, :],
                                                  acc[irt][:rl, :],
                                                  sl, op=ADD)

              # ---------------- tail: transpose + Wo ----------------
              if True:
                with (
                    tc.tile_pool(name="tail", bufs=1) as tlp,
                    tc.tile_pool(name="tail2", bufs=2) as tlp2,
                    tc.tile_pool(name="c_ps", bufs=2, space="PSUM") as cps,
                    tc.tile_pool(name="o_ps", bufs=2, space="PSUM") as ops_,
                ):
                    opT = tlp.tile([128, CT * ROWS], F32R, tag="opT")
                    for irt, (ro, rl) in enumerate(RT):
                        for et in range(CT):
                            tp = cps.tile([128, 128], F32, tag="ctp")
                            nc.tensor.transpose(
                                tp[:, :rl],
                                acc[irt][:rl, et * 128:(et + 1) * 128],
                                ident[:rl, :rl])
                            nc.any.tensor_copy(
                                opT[:, et * ROWS + ro:et * ROWS + ro + rl],
                                tp[:, :rl])
                    for ft in range(CT):
                        for (co, cl) in RCHUNKS:
                            op_ps = ops_.tile([128, 512], F32, tag="ops")
                            for et in range(CT):
                                nc.tensor.matmul(
                                    op_ps[:, :cl],
                                    wo[:, et * D + ft * 128:
                                        et * D + ft * 128 + 128],
                                    opT[:, et * ROWS + co:
                                         et * ROWS + co + cl],
                                    start=(et == 0), stop=(et == CT - 1))
                            st = tlp2.tile([128, 512], F32, tag="cst")
                            nc.any.tensor_copy(st[:, :cl], op_ps[:, :cl])
                            nc.sync.dma_start(
                                outT_d[ft * 128:(ft + 1) * 128, co:co + cl],
                                st[:, :cl])
            _wv_cm.__exit__(None, None, None)

    nc.compile()
    return nc


_PROGRAM = None


def _get_program():
    global _PROGRAM
    if _PROGRAM is None:
        _PROGRAM = build_program()
    return _PROGRAM


def make_in_maps(x, context, Wq, Wk, Wv, Wo):
    x = np.ascontiguousarray(x, dtype=np.float32)
    context = np.ascontiguousarray(context, dtype=np.float32)
    xT = np.ascontiguousarray(x.reshape(ROWS, D).T)
    ctxT = np.ascontiguousarray(context.reshape(B * NK, D).T)
    Wq = np.ascontiguousarray(Wq, dtype=np.float32)
    Wk = np.ascontiguousarray(Wk, dtype=np.float32)
    Wo = np.ascontiguousarray(Wo, dtype=np.float32)
    in_maps = []
    for i in range(N_CORES):
        d1s = slice(i * DSH, (i + 1) * DSH)
        in_maps.append({
            "xT": xT,
            "ctxT": ctxT,
            "ctxd1": np.ascontiguousarray(
                context[:, :, d1s].reshape(B * NK, DSH)),
            "Wq": Wq,
            "Wk": Wk,
            "Wo": Wo,
            "Wv_s": np.ascontiguousarray(Wv[:, i * WCOLS:(i + 1) * WCOLS]),
        })
    return in_maps


def kernel(x, context, Wq, Wk, Wv, Wo):
    nc = _get_program()
    in_maps = make_in_maps(x, context, Wq, Wk, Wv, Wo)
    res = run_bass_kernel_spmd(nc, in_maps, list(range(N_CORES)))
    outT = np.zeros((D, ROWS), dtype=np.float64)
    for i in range(N_CORES):
        outT += res.results[i]["outT"].astype(np.float64)
    return np.ascontiguousarray(
        outT.T.reshape(B, NQ, D).astype(np.float32))

